# revision 14
# baseline (speedup 1.0000x reference)
"""Trainium2 Bass kernel for nn_Adapt_SIMLoss (loss_fn).

Math: with D = s_gt - fuse_fea (channels-major [3, HW] per batch) and
G in {gt0, gt1}, the loss is
    loss = sum_g w_g * mean_{n,p,q} | (D_n^T @ G_{g,n})[p,q] |
The 4 batches x 2 gt tensors give 8 independent partial sums -> one per
NeuronCore, data parallel, no collective (host adds 8 scalars).

Per-core pipeline:
  1. gating network (1x1 convs) channels-major on PE, softmax-over-2 as
     sigmoid of the logit difference, elementwise work pixel-major.
  2. D' = B*sigma - A (sign-flipped D; irrelevant under |.|),
     PE-transposed to channels-major.
  3. main loop: 256 matmul tiles [128,512] (K=3, fp32r) -> PSUM,
     consumed by fused abs+sum on ScalarE (activation Abs + accum_out)
     and VectorE (tensor_reduce apply_absolute_value) in a ~55/45 split.
  4. per-partition partials DMA'd out; host does the final tiny sum.
"""

import sys

for _p in ("/opt/pypackages", "/opt/trn_rl_repo"):
    if _p not in sys.path:
        sys.path.insert(0, _p)

import ml_dtypes
import numpy as np

N, C, H, W = 4, 3, 64, 64
HW = H * W                      # 4096
NBLK = HW // 128                # 32 p-blocks
NQ = HW // 512                  # 8 q-chunks of 512
NUNIT = NBLK * NQ // 4          # 64 units of 4 tiles (one 4-bank PSUM group)
ACT_UNITS = 35                  # ScalarE share of the 64 consumer units

_CACHED = {}


def _act_unit_set():
    # Bresenham-interleave ACT/DVE so both engines stay busy throughout.
    s = set()
    acc = 0
    for u in range(NUNIT):
        nxt = (u + 1) * ACT_UNITS // NUNIT
        if nxt > acc:
            s.add(u)
        acc = nxt
    return s


def _build_nc():
    from concourse import bacc, mybir
    from concourse import tile as tile_mod

    f32 = mybir.dt.float32
    bf16 = mybir.dt.bfloat16
    A = mybir.AluOpType
    AF = mybir.ActivationFunctionType
    AX = mybir.AxisListType

    nc = bacc.Bacc(None)

    p_F = nc.declare_dram_parameter("F", [7, HW], f32, isOutput=False)
    p_W1 = nc.declare_dram_parameter("W1", [7, 12], f32, isOutput=False)
    p_W2d = nc.declare_dram_parameter("W2d", [128, NBLK * 12], f32, isOutput=False)
    p_B2d = nc.declare_dram_parameter("B2d", [128, 1], f32, isOutput=False)
    p_S = nc.declare_dram_parameter("Spm", [128, 96], f32, isOutput=False)
    p_T = nc.declare_dram_parameter("Tpm", [128, 96], f32, isOutput=False)
    p_O = nc.declare_dram_parameter("Opm", [128, 96], f32, isOutput=False)
    p_G = nc.declare_dram_parameter("G", [3, HW], bf16, isOutput=False)
    p_I = nc.declare_dram_parameter("Ident", [128, 128], f32, isOutput=False)
    p_out = nc.declare_dram_parameter("out", [128, 2 * NUNIT], f32, isOutput=True)

    act_units = _act_unit_set()

    with tile_mod.TileContext(nc) as tc:
        with (
            tc.tile_pool(name="sb", bufs=1) as sb,
            tc.tile_pool(name="ps", bufs=2, space="PSUM") as ps,
        ):
            F_sb = sb.tile([7, HW], f32, tag="F")
            W1_sb = sb.tile([7, 12], f32, tag="W1")
            W2d_sb = sb.tile([128, NBLK * 12], f32, tag="W2d")
            B2d_sb = sb.tile([128, 1], f32, tag="B2d")
            S_sb = sb.tile([128, 96], f32, tag="S")
            T_sb = sb.tile([128, 96], f32, tag="T")
            O_sb = sb.tile([128, 96], f32, tag="O")
            G_sb = sb.tile([3, HW], bf16, tag="G")
            I_sb = sb.tile([128, 128], f32, tag="I")

            for t, p in (
                (F_sb, p_F), (W1_sb, p_W1), (W2d_sb, p_W2d), (B2d_sb, p_B2d),
                (S_sb, p_S), (T_sb, p_T), (O_sb, p_O), (G_sb, p_G), (I_sb, p_I),
            ):
                nc.sync.dma_start(t[:, :], p[:, :])

            # ---- gating network ----
            # conv1 (channels-major): h^T blocks [128pix, 12] via K=7 matmuls
            # (6 fusion channels + ones row folds in the bias).
            psg = ps.tile([128, 2048], f32, tag="mm")
            for b in range(NBLK):
                nc.tensor.matmul(
                    psg[:, b * 12:(b + 1) * 12],
                    lhsT=F_sb[:, b * 128:(b + 1) * 128],
                    rhs=W1_sb[:, :],
                )
            hT = sb.tile([128, NBLK * 12], f32, tag="hT")
            nc.scalar.activation(hT[:, :], psg[:, 0:NBLK * 12], AF.Relu)

            # conv2 as broadcast-mult + reduce over the 12 hidden channels.
            prod = sb.tile([128, NBLK * 12], f32, tag="prod")
            nc.vector.tensor_mul(prod[:, :], hT[:, :], W2d_sb[:, :])
            diff = sb.tile([128, NBLK], f32, tag="diff")
            nc.vector.tensor_reduce(
                diff[:, :],
                prod[:, :].rearrange("p (b c) -> p b c", c=12),
                axis=AX.X,
                op=A.add,
            )
            score = sb.tile([128, NBLK], f32, tag="score")
            nc.scalar.activation(
                score[:, :], diff[:, :], AF.Sigmoid, bias=B2d_sb[:, 0:1]
            )

            # D' = (t_gt - t_gtout)*sigma - (s_gt - t_gtout), pixel-major.
            Bt = sb.tile([128, 96], f32, tag="Bt")
            nc.vector.tensor_sub(Bt[:, :], T_sb[:, :], O_sb[:, :])
            At = sb.tile([128, 96], f32, tag="At")
            nc.vector.tensor_sub(At[:, :], S_sb[:, :], O_sb[:, :])
            Dpm = sb.tile([128, 96], f32, tag="Dpm")
            for c in range(3):
                cs = slice(c * 32, (c + 1) * 32)
                nc.vector.scalar_tensor_tensor(
                    Dpm[:, cs], Bt[:, cs], 0.0, score[:, :],
                    op0=A.bypass, op1=A.mult,
                )
                nc.vector.tensor_sub(Dpm[:, cs], Dpm[:, cs], At[:, cs])

            # channels-major D' via PE transpose: [128,96] -> [96,128]
            pst = ps.tile([128, 2048], f32, tag="mm")
            nc.tensor.transpose(pst[0:96, 0:128], Dpm[:, :], I_sb[:, :])
            DT = sb.tile([96, 128], bf16, tag="DT")
            nc.scalar.copy(DT[:, :], pst[0:96, 0:128])
            # collapse (c*32+b, p) partitions -> channels-major [3, HW]
            # (row block c*32..c*32+31 read partition-major is exactly
            #  D'[c, b*128+p] in sequential order)
            Dcm = sb.tile([3, HW], bf16, tag="Dcm")
            for c in range(3):
                nc.sync.dma_start(
                    Dcm[c:c + 1, :], DT[c * 32:(c + 1) * 32, :]
                )

            # ---- main loop: sum |D'^T G| ----
            accA = sb.tile([128, NUNIT], f32, tag="accA")
            accV = sb.tile([128, NUNIT], f32, tag="accV")
            nc.vector.memset(accA[:, :], 0.0)
            nc.vector.memset(accV[:, :], 0.0)

            for u in range(NUNIT):
                pt = ps.tile([128, 2048], f32, tag="mm")
                b, qq = divmod(u, 2)
                lhs = Dcm[:, b * 128:(b + 1) * 128]
                for k in range(4):
                    q = qq * 4 + k
                    nc.tensor.matmul(
                        pt[:, k * 512:(k + 1) * 512],
                        lhsT=lhs,
                        rhs=G_sb[:, q * 512:(q + 1) * 512],
                    )
                if u in act_units:
                    nc.scalar.activation(
                        pt[:, :], pt[:, :], AF.Abs, accum_out=accA[:, u:u + 1]
                    )
                else:
                    nc.vector.tensor_reduce(
                        accV[:, u:u + 1], pt[:, :], axis=AX.X, op=A.add,
                        apply_absolute_value=True,
                    )

            nc.sync.dma_start(p_out[:, 0:NUNIT], accA[:, :])
            nc.sync.dma_start(p_out[:, NUNIT:2 * NUNIT], accV[:, :])

    nc.compile()
    return nc


def _shards(inputs):
    gt0 = np.asarray(inputs["gt0"], np.float32).reshape(N, C, HW)
    gt1 = np.asarray(inputs["gt1"], np.float32).reshape(N, C, HW)
    s_gt = np.asarray(inputs["s_gt"], np.float32).reshape(N, C, HW)
    t_gt = np.asarray(inputs["t_gt"], np.float32).reshape(N, C, HW)
    t_gtout = np.asarray(inputs["t_gtout"], np.float32).reshape(N, C, HW)
    w1 = np.asarray(inputs["w1"], np.float32)     # [12, 6]
    b1 = np.asarray(inputs["b1"], np.float32)     # [12]
    w2 = np.asarray(inputs["w2"], np.float32)     # [2, 12]
    b2 = np.asarray(inputs["b2"], np.float32)     # [2]

    W1a = np.ascontiguousarray(
        np.concatenate([w1.T, b1[None, :]], axis=0), np.float32
    )  # [7, 12]
    w2d = (w2[0] - w2[1]).astype(np.float32)      # [12]
    W2d = np.ascontiguousarray(np.tile(w2d, (128, NBLK)))   # [128, 384]
    B2d = np.full((128, 1), float(b2[0] - b2[1]), np.float32)
    ident = np.eye(128, dtype=np.float32)

    def pm(x):  # [3, HW] -> [128, 96] pixel-major, col = c*32 + b
        return np.ascontiguousarray(
            x.reshape(3, NBLK, 128).transpose(2, 0, 1).reshape(128, 96)
        )

    maps = []
    for i in range(8):
        n, g = i % 4, i // 4
        F = np.ascontiguousarray(np.concatenate(
            [t_gt[n], t_gtout[n], np.ones((1, HW), np.float32)], axis=0
        ))  # [7, HW]
        G = np.ascontiguousarray(
            (gt0 if g == 0 else gt1)[n].astype(ml_dtypes.bfloat16)
        )  # [3, HW]
        maps.append({
            "F": F, "W1": W1a, "W2d": W2d, "B2d": B2d,
            "Spm": pm(s_gt[n]), "Tpm": pm(t_gt[n]), "Opm": pm(t_gtout[n]),
            "G": G, "Ident": ident,
        })
    return maps


def _reduce_results(results):
    parts = [np.asarray(r["out"], np.float64).sum() for r in results]
    loss = (0.02 * sum(parts[:4]) + 1.0 * sum(parts[4:])) / (N * HW * HW)
    return np.float32(loss)


def _install_profile_hook():
    """The agent image's antenv lacks axon_hooks; inject a shim and
    register the ctypes NTFF hook so trace=True yields exec_time_ns."""
    import types

    try:
        import antenv.axon_hooks  # noqa: F401
        return
    except ImportError:
        pass
    mod = types.ModuleType("antenv.axon_hooks")
    mod._hook = None

    def set_axon_ntff_profile_hook(h):
        mod._hook = h

    def get_axon_ntff_profile_hook():
        return mod._hook

    mod.set_axon_ntff_profile_hook = set_axon_ntff_profile_hook
    mod.get_axon_ntff_profile_hook = get_axon_ntff_profile_hook
    import antenv

    sys.modules["antenv.axon_hooks"] = mod
    antenv.axon_hooks = mod
    try:
        from trn_agent_boot.trn_boot import _ntff_profile_via_ctypes

        mod._hook = _ntff_profile_via_ctypes("/opt/axon/libaxon_pjrt.so")
    except Exception as e:  # degrade: tracing skipped, run still works
        print(f"NTFF hook install failed: {e}", file=sys.stderr)


def _run(inputs, trace=False):
    from concourse.bass_utils import run_bass_kernel_spmd

    if trace:
        _install_profile_hook()

    if "nc" not in _CACHED:
        _CACHED["nc"] = _build_nc()
    nc = _CACHED["nc"]
    in_maps = _shards(inputs)
    res = run_bass_kernel_spmd(nc, in_maps, core_ids=list(range(8)), trace=trace)
    return _reduce_results(res.results), res


def kernel(**inputs) -> np.ndarray:
    loss, _ = _run(inputs, trace=False)
    return loss


def _simulate(inputs):
    """CoreSim-based local check (per-core, no hardware)."""
    from concourse.bass_interp import CoreSim

    nc = _build_nc()
    in_maps = _shards(inputs)
    results = []
    for i in range(8):
        sim = CoreSim(nc, trace=False)
        for k, v in in_maps[i].items():
            sim.tensor(k)[:] = v
        sim.simulate()
        results.append({"out": np.array(sim.tensor("out"))})
    return _reduce_results(results), results


# revision 19
# speedup vs baseline: 1.2337x; 1.2337x over previous
"""Trainium2 Bass kernel for nn_Adapt_SIMLoss (loss_fn).

Math: with D = s_gt - fuse_fea (channels-major [3, HW] per batch) and
G in {gt0, gt1}, the loss is
    loss = sum_g w_g * mean_{n,p,q} | (D_n^T @ G_{g,n})[p,q] |
The 4 batches x 2 gt tensors give 8 independent partial sums -> one per
NeuronCore, data parallel, no collective (host adds 8 scalars).

Per-core pipeline:
  1. gating network (1x1 convs) channels-major on PE, softmax-over-2 as
     sigmoid of the logit difference, elementwise work pixel-major.
  2. D' = B*sigma - A (sign-flipped D; irrelevant under |.|),
     PE-transposed to channels-major.
  3. main loop: 256 matmul tiles [128,512] (K=3, fp32r) -> PSUM,
     consumed by fused abs+sum on ScalarE (activation Abs + accum_out)
     and VectorE (tensor_reduce apply_absolute_value) in a ~55/45 split.
  4. per-partition partials DMA'd out; host does the final tiny sum.
"""

import sys

for _p in ("/opt/pypackages", "/opt/trn_rl_repo"):
    if _p not in sys.path:
        sys.path.insert(0, _p)

import ml_dtypes
import numpy as np

N, C, H, W = 4, 3, 64, 64
HW = H * W                      # 4096
NBLK = HW // 128                # 32 p-blocks
NQ = HW // 512                  # 8 q-chunks of 512
NUNIT = NBLK * NQ // 4          # 64 units of 4 tiles (one 4-bank PSUM group)
ACT_UNITS = 33                  # ScalarE share of the 64 consumer units

_CACHED = {}


def _act_unit_set():
    # Bresenham-interleave ACT/DVE so both engines stay busy throughout.
    s = set()
    acc = 0
    for u in range(NUNIT):
        nxt = (u + 1) * ACT_UNITS // NUNIT
        if nxt > acc:
            s.add(u)
        acc = nxt
    return s


def _build_nc():
    from concourse import bacc, mybir
    from concourse import tile as tile_mod

    f32 = mybir.dt.float32
    bf16 = mybir.dt.bfloat16
    A = mybir.AluOpType
    AF = mybir.ActivationFunctionType
    AX = mybir.AxisListType

    nc = bacc.Bacc(None)

    p_F = nc.declare_dram_parameter("F", [7, HW], f32, isOutput=False)
    p_W1 = nc.declare_dram_parameter("W1", [7, 12], f32, isOutput=False)
    p_W2d = nc.declare_dram_parameter("W2d", [128, NBLK * 12], f32, isOutput=False)
    p_B2d = nc.declare_dram_parameter("B2d", [128, 1], f32, isOutput=False)
    p_S = nc.declare_dram_parameter("Spm", [128, 96], f32, isOutput=False)
    p_T = nc.declare_dram_parameter("Tpm", [128, 96], f32, isOutput=False)
    p_O = nc.declare_dram_parameter("Opm", [128, 96], f32, isOutput=False)
    p_G = nc.declare_dram_parameter("G", [3, HW], bf16, isOutput=False)
    p_I = nc.declare_dram_parameter("Ident", [128, 128], f32, isOutput=False)
    p_out = nc.declare_dram_parameter("out", [128, 2 * NUNIT], f32, isOutput=True)

    act_units = _act_unit_set()

    with tile_mod.TileContext(nc) as tc:
        with (
            tc.tile_pool(name="sb", bufs=1) as sb,
            tc.tile_pool(name="ps", bufs=2, space="PSUM") as ps,
        ):
            F_sb = sb.tile([128, HW // 4], f32, tag="F")
            W1_sb = sb.tile([128, 12], f32, tag="W1")
            W2d_sb = sb.tile([128, NBLK * 12], f32, tag="W2d")
            B2d_sb = sb.tile([128, 1], f32, tag="B2d")
            S_sb = sb.tile([128, 96], f32, tag="S")
            T_sb = sb.tile([128, 96], f32, tag="T")
            O_sb = sb.tile([128, 96], f32, tag="O")
            G_sb = sb.tile([128, HW], bf16, tag="G")
            I_sb = sb.tile([128, 128], f32, tag="I")

            for t, p in (
                (W2d_sb, p_W2d), (B2d_sb, p_B2d),
                (S_sb, p_S), (T_sb, p_T), (O_sb, p_O), (I_sb, p_I),
            ):
                nc.sync.dma_start(t[:, :], p[:, :])
            # replicate at partition offsets 0/32/64/96 for 4x row tiling
            for g in range(4):
                nc.sync.dma_start(
                    F_sb[32 * g:32 * g + 7, :], p_F[:, g * 1024:(g + 1) * 1024]
                )
                nc.sync.dma_start(W1_sb[32 * g:32 * g + 7, :], p_W1[:, :])
                nc.sync.dma_start(G_sb[32 * g:32 * g + 3, :], p_G[:, :])

            # dummy sigmoid first: pin the act-table set (contains
            # relu/abs/copy as fillers) so only one ACT_TABLE_LOAD happens,
            # overlapped with the input DMAs.
            scr = sb.tile([128, 1], f32, tag="scr")
            nc.scalar.activation(scr[:, :], B2d_sb[:, 0:1], AF.Sigmoid)

            # ---- gating network ----
            # conv1 (channels-major): h^T blocks [128pix, 12] via K=7 matmuls
            # (6 fusion channels + ones row folds in the bias), 4x row-tiled.
            psg = ps.tile([128, 2048], f32, tag="mm")
            for g in range(4):
                for j in range(8):
                    nc.tensor.matmul(
                        psg[:, g * 512 + j * 12:g * 512 + (j + 1) * 12],
                        lhsT=F_sb[32 * g:32 * g + 7, j * 128:(j + 1) * 128],
                        rhs=W1_sb[32 * g:32 * g + 7, :],
                        tile_position=(32 * g, 0),
                    )
            hT = sb.tile([128, NBLK * 12], f32, tag="hT")
            nc.scalar.activation(
                hT[:, :].rearrange("p (g x) -> p g x", g=4),
                psg[:, :].rearrange("p (g x) -> p g x", g=4)[:, :, 0:96],
                AF.Relu,
            )

            # conv2 as broadcast-mult + reduce over the 12 hidden channels.
            prod = sb.tile([128, NBLK * 12], f32, tag="prod")
            nc.vector.tensor_mul(prod[:, :], hT[:, :], W2d_sb[:, :])
            diff = sb.tile([128, NBLK], f32, tag="diff")
            nc.vector.tensor_reduce(
                diff[:, :],
                prod[:, :].rearrange("p (b c) -> p b c", c=12),
                axis=AX.X,
                op=A.add,
            )
            score = sb.tile([128, NBLK], f32, tag="score")
            nc.scalar.activation(
                score[:, :], diff[:, :], AF.Sigmoid, bias=B2d_sb[:, 0:1]
            )

            # D' = (t_gt - t_gtout)*sigma - (s_gt - t_gtout), pixel-major.
            Bt = sb.tile([128, 96], f32, tag="Bt")
            nc.vector.tensor_sub(Bt[:, :], T_sb[:, :], O_sb[:, :])
            At = sb.tile([128, 96], f32, tag="At")
            nc.vector.tensor_sub(At[:, :], S_sb[:, :], O_sb[:, :])
            Dpm = sb.tile([128, 96], f32, tag="Dpm")
            for c in range(3):
                cs = slice(c * 32, (c + 1) * 32)
                nc.vector.scalar_tensor_tensor(
                    Dpm[:, cs], Bt[:, cs], 0.0, score[:, :],
                    op0=A.bypass, op1=A.mult,
                )
                nc.vector.tensor_sub(Dpm[:, cs], Dpm[:, cs], At[:, cs])

            # channels-major D' via PE transpose: [128,96] -> [96,128]
            pst = ps.tile([128, 2048], f32, tag="mm")
            nc.tensor.transpose(pst[0:96, 0:128], Dpm[:, :], I_sb[:, :])
            DT = sb.tile([96, 128], bf16, tag="DT")
            nc.scalar.copy(DT[:, :], pst[0:96, 0:128])
            # collapse (c*32+b, p) partitions -> channels-major [3, HW]
            # (row block c*32..c*32+31 read partition-major is exactly
            #  D'[c, b*128+p] in sequential order), replicated at the 4
            # row-tiling partition offsets
            Dcm = sb.tile([128, HW], bf16, tag="Dcm")
            for off in (0, 32, 64, 96):
                for c in range(3):
                    nc.sync.dma_start(
                        Dcm[off + c:off + c + 1, :], DT[c * 32:(c + 1) * 32, :]
                    )

            # ---- main loop: sum |D'^T G| ----
            accA = sb.tile([128, NUNIT], f32, tag="accA")
            accV = sb.tile([128, NUNIT], f32, tag="accV")
            nc.vector.memset(accA[:, :], 0.0)
            nc.vector.memset(accV[:, :], 0.0)

            for u in range(NUNIT):
                pt = ps.tile([128, 2048], f32, tag="mm")
                b, qq = divmod(u, 2)
                for k in range(4):
                    q = qq * 4 + k
                    nc.tensor.matmul(
                        pt[:, k * 512:(k + 1) * 512],
                        lhsT=Dcm[32 * k:32 * k + 3, b * 128:(b + 1) * 128],
                        rhs=G_sb[32 * k:32 * k + 3, q * 512:(q + 1) * 512],
                        tile_position=(32 * k, 0),
                    )
                if u in act_units:
                    nc.scalar.activation(
                        pt[:, :], pt[:, :], AF.Abs, accum_out=accA[:, u:u + 1]
                    )
                else:
                    nc.vector.tensor_reduce(
                        accV[:, u:u + 1], pt[:, :], axis=AX.X, op=A.add,
                        apply_absolute_value=True,
                    )

            nc.sync.dma_start(p_out[:, 0:NUNIT], accA[:, :])
            nc.sync.dma_start(p_out[:, NUNIT:2 * NUNIT], accV[:, :])

    nc.compile()
    return nc


def _shards(inputs):
    gt0 = np.asarray(inputs["gt0"], np.float32).reshape(N, C, HW)
    gt1 = np.asarray(inputs["gt1"], np.float32).reshape(N, C, HW)
    s_gt = np.asarray(inputs["s_gt"], np.float32).reshape(N, C, HW)
    t_gt = np.asarray(inputs["t_gt"], np.float32).reshape(N, C, HW)
    t_gtout = np.asarray(inputs["t_gtout"], np.float32).reshape(N, C, HW)
    w1 = np.asarray(inputs["w1"], np.float32)     # [12, 6]
    b1 = np.asarray(inputs["b1"], np.float32)     # [12]
    w2 = np.asarray(inputs["w2"], np.float32)     # [2, 12]
    b2 = np.asarray(inputs["b2"], np.float32)     # [2]

    W1a = np.ascontiguousarray(
        np.concatenate([w1.T, b1[None, :]], axis=0), np.float32
    )  # [7, 12]
    w2d = (w2[0] - w2[1]).astype(np.float32)      # [12]
    W2d = np.ascontiguousarray(np.tile(w2d, (128, NBLK)))   # [128, 384]
    B2d = np.full((128, 1), float(b2[0] - b2[1]), np.float32)
    ident = np.eye(128, dtype=np.float32)

    def pm(x):  # [3, HW] -> [128, 96] pixel-major, col = c*32 + b
        return np.ascontiguousarray(
            x.reshape(3, NBLK, 128).transpose(2, 0, 1).reshape(128, 96)
        )

    maps = []
    for i in range(8):
        n, g = i % 4, i // 4
        F = np.ascontiguousarray(np.concatenate(
            [t_gt[n], t_gtout[n], np.ones((1, HW), np.float32)], axis=0
        ))  # [7, HW]
        G = np.ascontiguousarray(
            (gt0 if g == 0 else gt1)[n].astype(ml_dtypes.bfloat16)
        )  # [3, HW]
        maps.append({
            "F": F, "W1": W1a, "W2d": W2d, "B2d": B2d,
            "Spm": pm(s_gt[n]), "Tpm": pm(t_gt[n]), "Opm": pm(t_gtout[n]),
            "G": G, "Ident": ident,
        })
    return maps


def _reduce_results(results):
    parts = [np.asarray(r["out"], np.float64).sum() for r in results]
    loss = (0.02 * sum(parts[:4]) + 1.0 * sum(parts[4:])) / (N * HW * HW)
    return np.float32(loss)


def _install_profile_hook():
    """The agent image's antenv lacks axon_hooks; inject a shim and
    register the ctypes NTFF hook so trace=True yields exec_time_ns."""
    import types

    try:
        import antenv.axon_hooks  # noqa: F401
        return
    except ImportError:
        pass
    mod = types.ModuleType("antenv.axon_hooks")
    mod._hook = None

    def set_axon_ntff_profile_hook(h):
        mod._hook = h

    def get_axon_ntff_profile_hook():
        return mod._hook

    mod.set_axon_ntff_profile_hook = set_axon_ntff_profile_hook
    mod.get_axon_ntff_profile_hook = get_axon_ntff_profile_hook
    import antenv

    sys.modules["antenv.axon_hooks"] = mod
    antenv.axon_hooks = mod
    try:
        from trn_agent_boot.trn_boot import _ntff_profile_via_ctypes

        mod._hook = _ntff_profile_via_ctypes("/opt/axon/libaxon_pjrt.so")
    except Exception as e:  # degrade: tracing skipped, run still works
        print(f"NTFF hook install failed: {e}", file=sys.stderr)


def _run(inputs, trace=False):
    from concourse.bass_utils import run_bass_kernel_spmd

    if trace:
        _install_profile_hook()

    if "nc" not in _CACHED:
        _CACHED["nc"] = _build_nc()
    nc = _CACHED["nc"]
    in_maps = _shards(inputs)
    res = run_bass_kernel_spmd(nc, in_maps, core_ids=list(range(8)), trace=trace)
    return _reduce_results(res.results), res


def kernel(**inputs) -> np.ndarray:
    loss, _ = _run(inputs, trace=False)
    return loss


def _simulate(inputs):
    """CoreSim-based local check (per-core, no hardware)."""
    from concourse.bass_interp import CoreSim

    nc = _build_nc()
    in_maps = _shards(inputs)
    results = []
    for i in range(8):
        sim = CoreSim(nc, trace=False)
        for k, v in in_maps[i].items():
            sim.tensor(k)[:] = v
        sim.simulate()
        results.append({"out": np.array(sim.tensor("out"))})
    return _reduce_results(results), results
